# revision 3
# baseline (speedup 1.0000x reference)
"""Trainium2 Bass kernel for nn_Encoding (vq_codebook soft-assignment encoding).

Reference computation (per batch b):
    xf   = x[b].reshape(C, N).T                      # [N, C],  N = H*W
    x2_n = sum_c xf[n,c]^2
    xc   = xf @ codewords.T                          # [N, K]
    dist = scale_k * (x2_n - 2*xc + c2_k)            # [N, K]
    w    = softmax_k(dist)                           # [N, K]
    out  = w.T @ xf - (sum_n w)[:,None]*codewords    # [K, C]

Strategy: data-parallel over B across 8 cores (4 batches/core). The host feeds
x twice in fp16 — once channel-major [C, N] (for the xc matmul, contraction
over C) and once position-major [N, C] (for x2 and the w-aggregation matmul,
contraction over N) — because the tensor engine contracts over the partition
axis and an on-chip fp32 transpose of 8MB/batch is far more expensive than a
second DMA read at half precision. All matmuls are fp16 with fp32 PSUM
accumulation (measured end-to-end scale-relative absmax error ~3e-4).
Softmax skips the max-subtraction: dist <= 0 always (scale <= 0), and the
per-n max of dist is far from fp32 underflow for these inputs.
"""

import numpy as np

B, C, H, W = 32, 512, 64, 64
N = H * W          # 4096
K = 32
NCORES = 8
BPC = B // NCORES  # batches per core
CT = C // 128      # 4 c-tiles
NT = N // 128      # 32 n-tiles

_compiled = {}


def _build():
    import concourse.bacc as bacc
    import concourse.tile as tile
    from concourse import mybir

    f16 = mybir.dt.float16
    f32 = mybir.dt.float32
    Alu = mybir.AluOpType
    Act = mybir.ActivationFunctionType

    # Bacc (not plain Bass): its compile() pipeline splits multi-sem waits
    # into event-semaphore chains — the TRN2 ISA allows one wait per
    # instruction, and walrus rejects the raw multi-wait form Tile emits.
    nc = bacc.Bacc()
    x_cn = nc.dram_tensor("x_cn", [BPC, 128, CT, N], f16, kind="ExternalInput")
    x_nc = nc.dram_tensor("x_nc", [BPC, 128, NT, C], f16, kind="ExternalInput")
    cwt = nc.dram_tensor("cwt", [128, CT, K], f16, kind="ExternalInput")
    c2rep = nc.dram_tensor("c2rep", [128, K], f32, kind="ExternalInput")
    srep = nc.dram_tensor("srep", [128, K], f32, kind="ExternalInput")
    cwf = nc.dram_tensor("cwf", [K, C], f32, kind="ExternalInput")
    out = nc.dram_tensor("out", [BPC, K, C], f32, kind="ExternalOutput")

    with tile.TileContext(nc) as tc:
        with (
            tc.tile_pool(name="consts", bufs=1) as consts,
            tc.tile_pool(name="xcn", bufs=2) as xcn_pool,
            tc.tile_pool(name="xnc", bufs=2) as xnc_pool,
            tc.tile_pool(name="scr", bufs=2) as scr_pool,
            tc.tile_pool(name="small", bufs=4) as small,
            tc.tile_pool(name="ost", bufs=2) as ost,
            tc.tile_pool(name="psd", bufs=2, space="PSUM") as psd,
            tc.tile_pool(name="pswx", bufs=2, space="PSUM") as pswx,
            tc.tile_pool(name="psws", bufs=2, space="PSUM") as psws,
        ):
            cwt_sb = consts.tile([128, CT, K], f16)
            nc.sync.dma_start(cwt_sb[:], cwt[:])
            c2_sb = consts.tile([128, K], f32)
            nc.sync.dma_start(c2_sb[:], c2rep[:])
            s_sb = consts.tile([128, K], f32)
            nc.sync.dma_start(s_sb[:], srep[:])
            cwf_sb = consts.tile([K, C], f32)
            nc.sync.dma_start(cwf_sb[:], cwf[:])
            ones16 = consts.tile([128, 1], f16)
            nc.vector.memset(ones16[:], 1.0)

            for b in range(BPC):
                xnc_sb = xnc_pool.tile([128, NT, C], f16)
                nc.sync.dma_start(xnc_sb[:], x_nc[b])
                xcn_sb = xcn_pool.tile([128, CT, N], f16)
                nc.sync.dma_start(xcn_sb[:], x_cn[b])

                wx = pswx.tile([K, C], f32)
                ws = psws.tile([K, 1], f32)

                for nt in range(NT):
                    # x2 for this n-tile: ACT square + free-dim accumulate
                    scratch = scr_pool.tile([128, C], f16, tag="scr")
                    x2 = small.tile([128, 1], f32, tag="x2")
                    nc.scalar.activation(
                        scratch[:], xnc_sb[:, nt, :], Act.Square, accum_out=x2[:]
                    )

                    # xc accumulation over c-tiles: psum = xf_tile @ cw^T
                    dist = psd.tile([128, K], f32)
                    for ct in range(CT):
                        nc.tensor.matmul(
                            dist[:],
                            xcn_sb[:, ct, nt * 128 : (nt + 1) * 128],
                            cwt_sb[:, ct, :],
                            start=(ct == 0),
                            stop=(ct == CT - 1),
                        )

                    # dist = s_k * ((x2 - 2*xc) + c2_k)
                    u = small.tile([128, K], f32, tag="u")
                    nc.vector.tensor_scalar(
                        u[:], dist[:], -2.0, x2[:], Alu.mult, Alu.add
                    )
                    v = small.tile([128, K], f32, tag="v")
                    nc.vector.tensor_tensor(v[:], u[:], c2_sb[:], Alu.add)
                    d2 = small.tile([128, K], f32, tag="d2")
                    nc.vector.tensor_tensor(d2[:], v[:], s_sb[:], Alu.mult)

                    # softmax over k (free dim); dist<=0 so exp never overflows
                    e = small.tile([128, K], f16, tag="e")
                    esum = small.tile([128, 1], f32, tag="esum")
                    nc.scalar.activation(e[:], d2[:], Act.Exp, accum_out=esum[:])
                    r = small.tile([128, 1], f32, tag="r")
                    nc.vector.reciprocal(r[:], esum[:])
                    wt = small.tile([128, K], f16, tag="wt")
                    nc.vector.tensor_scalar(wt[:], e[:], r[:], None, Alu.mult)

                    # aggregate: wx += w^T @ xf ; wsum += w^T @ 1
                    nc.tensor.matmul(
                        wx[:], wt[:], xnc_sb[:, nt, :],
                        start=(nt == 0), stop=(nt == NT - 1),
                    )
                    nc.tensor.matmul(
                        ws[:], wt[:], ones16[:],
                        start=(nt == 0), stop=(nt == NT - 1),
                    )

                # out = wx - wsum * codewords
                ws_sb = small.tile([K, 1], f32, tag="ws_sb")
                nc.vector.tensor_copy(ws_sb[:], ws[:])
                tmp = ost.tile([K, C], f32, tag="tmp")
                nc.vector.tensor_scalar(tmp[:], cwf_sb[:], ws_sb[:], None, Alu.mult)
                o_sb = ost.tile([K, C], f32, tag="o_sb")
                nc.vector.tensor_tensor(o_sb[:], wx[:], tmp[:], Alu.subtract)
                nc.sync.dma_start(out[b], o_sb[:])

    nc.compile()
    return nc


def _prep_inputs(x, codewords, scale):
    """Host-side shard + layout prep. Returns list of 8 per-core input maps."""
    xr = np.ascontiguousarray(x.reshape(B, C, N))
    c2 = (codewords.astype(np.float64) ** 2).sum(axis=1).astype(np.float32)

    cwt = np.ascontiguousarray(
        codewords.T.reshape(CT, 128, K).transpose(1, 0, 2)
    ).astype(np.float16)                                   # [128, CT, K]
    c2rep = np.broadcast_to(c2[None, :], (128, K)).copy()  # [128, K] f32
    srep = np.broadcast_to(scale[None, :], (128, K)).astype(np.float32).copy()
    cwf = codewords.astype(np.float32)

    in_maps = []
    for core in range(NCORES):
        xb = xr[core * BPC : (core + 1) * BPC]             # [BPC, C, N] f32
        x16 = xb.astype(np.float16)
        # [b, p, ct, n] with channel = ct*128 + p
        x_cn = np.ascontiguousarray(
            x16.reshape(BPC, CT, 128, N).transpose(0, 2, 1, 3)
        )
        # [b, p, nt, c] with position = nt*128 + p
        xt = np.ascontiguousarray(x16.transpose(0, 2, 1))  # [BPC, N, C]
        x_nc = np.ascontiguousarray(
            xt.reshape(BPC, NT, 128, C).transpose(0, 2, 1, 3)
        )
        in_maps.append(
            {
                "x_cn": x_cn,
                "x_nc": x_nc,
                "cwt": cwt,
                "c2rep": c2rep,
                "srep": srep,
                "cwf": cwf,
            }
        )
    return in_maps


def kernel(x, codewords, scale, _trace=False):
    from concourse.bass_utils import run_bass_kernel_spmd

    x = np.asarray(x, dtype=np.float32)
    codewords = np.asarray(codewords, dtype=np.float32)
    scale = np.asarray(scale, dtype=np.float32)

    if "nc" not in _compiled:
        _compiled["nc"] = _build()
    nc = _compiled["nc"]

    in_maps = _prep_inputs(x, codewords, scale)
    res = run_bass_kernel_spmd(
        nc, in_maps, core_ids=list(range(NCORES)), trace=_trace
    )
    outs = [res.results[c]["out"] for c in range(NCORES)]
    full = np.concatenate(outs, axis=0).astype(np.float32)  # [B, K, C]
    if _trace:
        _compiled["last_results"] = res
    return full


# revision 9
# speedup vs baseline: 1.3014x; 1.3014x over previous
"""Trainium2 Bass kernel for nn_Encoding (vq_codebook soft-assignment encoding).

Reference computation (per batch b):
    xf   = x[b].reshape(C, N).T                      # [N, C],  N = H*W
    x2_n = sum_c xf[n,c]^2
    xc   = xf @ codewords.T                          # [N, K]
    dist = scale_k * (x2_n - 2*xc + c2_k)            # [N, K]
    w    = softmax_k(dist)                           # [N, K]
    out  = w.T @ xf - (sum_n w)[:,None]*codewords    # [K, C]

Strategy: data-parallel over B across 8 cores (4 batches/core). The host feeds
x twice in fp16 — once channel-major [C, N] (for the xc matmul, contraction
over C) and once position-major [N, C] (for x2 and the w-aggregation matmul,
contraction over N) — because the tensor engine contracts over the partition
axis and an on-chip fp32 transpose of 8MB/batch is far more expensive than a
second DMA read at half precision. All matmuls are fp16 with fp32 PSUM
accumulation (measured end-to-end scale-relative absmax error ~3e-4).
Softmax skips the max-subtraction: dist <= 0 always (scale <= 0), and the
per-n max of dist is far from fp32 underflow for these inputs.
"""

import numpy as np

B, C, H, W = 32, 512, 64, 64
N = H * W          # 4096
K = 32
NCORES = 8
BPC = B // NCORES  # batches per core
CT = C // 128      # 4 c-tiles
NT = N // 128      # 32 n-tiles

_compiled = {}


def _build():
    import concourse.bacc as bacc
    import concourse.tile as tile
    from concourse import mybir

    f16 = mybir.dt.float16
    f32 = mybir.dt.float32
    Alu = mybir.AluOpType
    Act = mybir.ActivationFunctionType

    # Bacc (not plain Bass): its compile() pipeline splits multi-sem waits
    # into event-semaphore chains — the TRN2 ISA allows one wait per
    # instruction, and walrus rejects the raw multi-wait form Tile emits.
    nc = bacc.Bacc()
    x_cn = nc.dram_tensor("x_cn", [BPC, 128, CT, N], f16, kind="ExternalInput")
    x_nc = nc.dram_tensor("x_nc", [BPC, 128, NT, C], f16, kind="ExternalInput")
    cwt = nc.dram_tensor("cwt", [128, CT, K], f16, kind="ExternalInput")
    c2row = nc.dram_tensor("c2row", [1, K], f16, kind="ExternalInput")
    srep = nc.dram_tensor("srep", [128, K], f32, kind="ExternalInput")
    cwf = nc.dram_tensor("cwf", [K, C], f32, kind="ExternalInput")
    out = nc.dram_tensor("out", [BPC, K, C], f32, kind="ExternalOutput")

    with tile.TileContext(nc) as tc:
        with (
            tc.tile_pool(name="consts", bufs=1) as consts,
            tc.tile_pool(name="xcn", bufs=2) as xcn_pool,
            tc.tile_pool(name="xnc", bufs=2) as xnc_pool,
            tc.tile_pool(name="scr", bufs=4) as scr_pool,
            tc.tile_pool(name="small", bufs=8) as small,
            tc.tile_pool(name="ost", bufs=2) as ost,
            tc.tile_pool(name="psd", bufs=3, space="PSUM") as psd,
            tc.tile_pool(name="pswx", bufs=2, space="PSUM") as pswx,
            tc.tile_pool(name="psws", bufs=2, space="PSUM") as psws,
        ):
            G = 8            # n-tiles per softmax group (batched small ops)
            NG = NT // G     # groups per batch

            cwt_sb = consts.tile([128, CT, K], f16)
            nc.sync.dma_start(cwt_sb[:], cwt[:])
            c2_sb = consts.tile([1, K], f16)
            nc.sync.dma_start(c2_sb[:], c2row[:])
            s_sb = consts.tile([128, K], f32)
            nc.sync.dma_start(s_sb[:], srep[:])
            cwf_sb = consts.tile([K, C], f32)
            nc.sync.dma_start(cwf_sb[:], cwf[:])
            ones16 = consts.tile([128, 1], f16)
            nc.vector.memset(ones16[:], 1.0)
            onesrow = consts.tile([1, 128], f16)
            nc.vector.memset(onesrow[:], 1.0)

            for b in range(BPC):
                xnc_sb = xnc_pool.tile([128, NT, C], f16)
                nc.sync.dma_start(xnc_sb[:], x_nc[b])
                xcn_sb = xcn_pool.tile([128, CT, N], f16)
                nc.sync.dma_start(xcn_sb[:], x_cn[b])

                wx = pswx.tile([K, C], f32)
                ws = psws.tile([K, 1], f32)

                for g in range(NG):
                    # x2 per n-tile: ACT square + free-dim accumulate
                    x2g = small.tile([128, G], f32, tag="x2g")
                    for j in range(G):
                        nt = g * G + j
                        scratch = scr_pool.tile([128, C], f16, tag="scr")
                        nc.scalar.activation(
                            scratch[:], xnc_sb[:, nt, :], Act.Square,
                            accum_out=x2g[:, j : j + 1],
                        )

                    # psum[:, j, :] = -2*xc + c2_k  (c2 via rank-1 ones matmul)
                    dist = psd.tile([128, G, K], f32)
                    for j in range(G):
                        nt = g * G + j
                        for ct in range(CT):
                            nc.tensor.matmul(
                                dist[:, j, :],
                                xcn_sb[:, ct, nt * 128 : (nt + 1) * 128],
                                cwt_sb[:, ct, :],
                                start=(ct == 0),
                                stop=False,
                            )
                        nc.tensor.matmul(
                            dist[:, j, :], onesrow[:], c2_sb[:],
                            start=False, stop=True,
                        )

                    # dist = s_k * (psum + x2_n); exp; normalize — all batched
                    u = small.tile([128, G, K], f32, tag="u")
                    nc.vector.tensor_tensor(
                        u[:], dist[:], x2g[:].broadcast_to((128, G, K)), Alu.add
                    )
                    d2 = small.tile([128, G, K], f32, tag="d2")
                    nc.vector.tensor_tensor(
                        d2[:], u[:],
                        s_sb[:].rearrange("p (o k) -> p o k", o=1).broadcast_to(
                            (128, G, K)
                        ),
                        Alu.mult,
                    )
                    e = small.tile([128, G, K], f32, tag="e")
                    nc.scalar.activation(e[:], d2[:], Act.Exp)
                    esum = small.tile([128, G], f32, tag="esum")
                    nc.vector.tensor_reduce(
                        esum[:], e[:], mybir.AxisListType.X, Alu.add
                    )
                    r = small.tile([128, G], f32, tag="r")
                    nc.vector.reciprocal(r[:], esum[:])
                    wt = small.tile([128, G, K], f16, tag="wt")
                    nc.vector.tensor_tensor(
                        wt[:], e[:], r[:].broadcast_to((128, G, K)), Alu.mult
                    )

                    # aggregate: wx += w^T @ xf ; wsum += w^T @ 1
                    for j in range(G):
                        nt = g * G + j
                        nc.tensor.matmul(
                            wx[:], wt[:, j, :], xnc_sb[:, nt, :],
                            start=(nt == 0), stop=(nt == NT - 1),
                        )
                        nc.tensor.matmul(
                            ws[:], wt[:, j, :], ones16[:],
                            start=(nt == 0), stop=(nt == NT - 1),
                        )

                # out = wx - wsum * codewords
                ws_sb = small.tile([K, 1], f32, tag="ws_sb")
                nc.vector.tensor_copy(ws_sb[:], ws[:])
                tmp = ost.tile([K, C], f32, tag="tmp")
                nc.vector.tensor_scalar(tmp[:], cwf_sb[:], ws_sb[:], None, Alu.mult)
                o_sb = ost.tile([K, C], f32, tag="o_sb")
                nc.vector.tensor_tensor(o_sb[:], wx[:], tmp[:], Alu.subtract)
                nc.sync.dma_start(out[b], o_sb[:])

    nc.compile()
    return nc


def _prep_inputs(x, codewords, scale):
    """Host-side shard + layout prep. Returns list of 8 per-core input maps."""
    xr = np.ascontiguousarray(x.reshape(B, C, N))
    c2 = (codewords.astype(np.float64) ** 2).sum(axis=1).astype(np.float32)

    # bake the -2 of (x2 - 2*xc + c2) into the matmul weights
    cwt = np.ascontiguousarray(
        (-2.0 * codewords.T).reshape(CT, 128, K).transpose(1, 0, 2)
    ).astype(np.float16)                                   # [128, CT, K]
    c2row = c2[None, :].astype(np.float16)                 # [1, K]
    srep = np.broadcast_to(scale[None, :], (128, K)).astype(np.float32).copy()
    cwf = codewords.astype(np.float32)

    in_maps = []
    for core in range(NCORES):
        xb = xr[core * BPC : (core + 1) * BPC]             # [BPC, C, N] f32
        x16 = xb.astype(np.float16)
        # [b, p, ct, n] with channel = ct*128 + p
        x_cn = np.ascontiguousarray(
            x16.reshape(BPC, CT, 128, N).transpose(0, 2, 1, 3)
        )
        # [b, p, nt, c] with position = nt*128 + p
        xt = np.ascontiguousarray(x16.transpose(0, 2, 1))  # [BPC, N, C]
        x_nc = np.ascontiguousarray(
            xt.reshape(BPC, NT, 128, C).transpose(0, 2, 1, 3)
        )
        in_maps.append(
            {
                "x_cn": x_cn,
                "x_nc": x_nc,
                "cwt": cwt,
                "c2row": c2row,
                "srep": srep,
                "cwf": cwf,
            }
        )
    return in_maps


def kernel(x, codewords, scale, _trace=False):
    from concourse.bass_utils import run_bass_kernel_spmd

    x = np.asarray(x, dtype=np.float32)
    codewords = np.asarray(codewords, dtype=np.float32)
    scale = np.asarray(scale, dtype=np.float32)

    if "nc" not in _compiled:
        _compiled["nc"] = _build()
    nc = _compiled["nc"]

    in_maps = _prep_inputs(x, codewords, scale)
    res = run_bass_kernel_spmd(
        nc, in_maps, core_ids=list(range(NCORES)), trace=_trace
    )
    outs = [res.results[c]["out"] for c in range(NCORES)]
    full = np.concatenate(outs, axis=0).astype(np.float32)  # [B, K, C]
    if _trace:
        _compiled["last_results"] = res
    return full
